# revision 14
# baseline (speedup 1.0000x reference)
"""GCN link-decoder kernel for 8 TRN2 NeuronCores (v2).

Math: both GCNConv layers are linear (no activation), so with
P = D^-1/2 (A+I) D^-1/2 the network output is
    value_e = sigmoid( h2[src_e] . h2[dst_e] ),  h2 = P^2 z W1 W2  (b1=b2=0)
which reduces to 16-dim aggregations:
    t0 = dinv * z;  agg1 = sum_{e into d} t0[src_e];  t1 = dinv^2*(agg1 + t0)
    agg2 = sum t1[src_e];  u = dinv*(agg2 + t1);  v = u @ G,  G = (W1W2)(W1W2)^T
    value_e = v[src_e] . u[dst_e]

v2 structure (vs v1): aggregation tables are stored as split-bf16 pairs
(hi|lo) so the scatter one-hot matmuls and their is_equal builds run at
bf16 DVE/PE rates while retaining ~fp32 accuracy; the t0 table is built
locally on every core from the full z input (no first AllGather); the t1
and u exchanges ship compact [npad,32|16] shards (3.2MB) instead of padded
256B-row tables (25.7MB); scoring runs on the src-owner core so v[src] is
gathered from the local v shard (never exchanged) and only u is
AllGathered; scoring math stays fp32 (the sigmoid tail needs ~2e-2
absolute logit accuracy).
"""
import sys
import os
import bisect
sys.path.insert(0, '/opt/trn_rl_repo')
import numpy as np
import ml_dtypes

NC = 8          # cores
P = 128         # partitions / chunk size
FW = 64         # f32 table row width (256B granule)
BW = 128        # bf16 table row width (256B granule)
BUCK = 32768    # int16 index bucket size (table rows per bucket)
BLK = 8192      # scoring gather idxs per dma_gather instruction
ABLK = 8192     # aggregation gather idxs per dma_gather instruction

bf16 = ml_dtypes.bfloat16


def _wrap_idx16(arr: np.ndarray) -> np.ndarray:
    """Linear int16 slot-index array (len % 128 == 0) -> [128, len/16] SWDGE
    wrapped layout (slot k at partition k%16, col k//16; 16-row pattern
    replicated to 128 partitions)."""
    n = arr.shape[0]
    t16 = arr.reshape(n // 16, 16).T
    return np.ascontiguousarray(np.tile(t16, (8, 1)))


def _host_reference(z, edge_index, W1, b1, W2, b2):
    """Numpy fallback (used only when b1/b2 are nonzero)."""
    N = z.shape[0]
    src, dst = edge_index[0], edge_index[1]
    deg = (np.bincount(dst, minlength=N) + 1.0).astype(np.float64)
    dinv = (1.0 / np.sqrt(deg)).astype(np.float32)

    def conv(x, W, b):
        h = x @ W
        out = np.zeros_like(h)
        np.add.at(out, dst, h[src] * (dinv[src] * dinv[dst])[:, None])
        out += h * (dinv * dinv)[:, None]
        return out + b

    h = conv(z, W1, b1)
    h = conv(h, W2, b2)
    val = np.einsum('ef,ef->e', h[src], h[dst]).astype(np.float64)
    return (1.0 / (1.0 + np.exp(-val))).astype(np.float32)


def _plan(z, edge_index):
    """Host-side layout planning: shard nodes/edges, build slot arrays."""
    N = z.shape[0]
    E = edge_index.shape[1]
    assert N % NC == 0 and E % NC == 0
    npc = N // NC                      # real nodes per core
    npad = ((npc + P - 1) // P) * P    # padded nodes per core
    tiles = npad // P
    nrows = NC * npad                  # table rows
    nbuck = (nrows + BUCK - 1) // BUCK

    src = edge_index[0].astype(np.int64)
    dst = edge_index[1].astype(np.int64)
    deg = np.bincount(dst, minlength=N).astype(np.float64) + 1.0
    dinv = (1.0 / np.sqrt(deg)).astype(np.float32)

    owner_s, local_s = src // npc, src % npc
    owner_d, local_d = dst // npc, dst % npc
    pid_s = (owner_s * npad + local_s).astype(np.int64)
    pid_d = (owner_d * npad + local_d).astype(np.int64)
    b_s = (pid_s // BUCK).astype(np.int64)
    b_d = (pid_d // BUCK).astype(np.int64)

    plan = {
        'N': N, 'E': E, 'npc': npc, 'npad': npad, 'tiles': tiles,
        'nrows': nrows, 'nbuck': nbuck, 'dinv': dinv,
    }

    # ---------------- aggregation slots (per dst-owner core) --------------
    # cell = (bucket(src), dst_tile); bucket-major order.
    t_d = local_d // P                     # dst tile within owner
    cell = b_s * tiles + t_d               # cell id within owner core
    ncell = nbuck * tiles
    counts = np.zeros((NC, ncell), np.int64)
    for c in range(NC):
        m = owner_d == c
        counts[c] = np.bincount(cell[m], minlength=ncell)
    K = np.maximum(np.ceil(counts.max(axis=0) / P).astype(np.int64), 0)
    cell_slots = K * P
    cell_ofs = np.concatenate([[0], np.cumsum(cell_slots)])
    tot_agg = int(cell_ofs[-1])
    plan['K'] = K
    plan['cell_ofs'] = cell_ofs
    plan['tot_agg'] = tot_agg
    plan['KMAX'] = max(int(K.max()), 1)

    agg_idx = np.zeros((NC, 128, tot_agg // 16), np.int16)
    agg_dstloc = np.full((NC, 128, tot_agg // 128), -1.0, np.float32)
    for c in range(NC):
        m = owner_d == c
        cl = cell[m]
        order = np.argsort(cl, kind='stable')
        cl_s = cl[order]
        grp_start = np.searchsorted(cl_s, np.arange(ncell))
        rank = np.arange(cl_s.shape[0]) - grp_start[cl_s]
        slot = cell_ofs[cl_s] + rank
        idx_lin = np.zeros(tot_agg, np.int16)
        dl_lin = np.full(tot_agg, -1.0, np.float32)
        ps = pid_s[m][order]
        idx_lin[slot] = (ps - (ps // BUCK) * BUCK).astype(np.int16)
        dl_lin[slot] = (local_d[m][order] % P).astype(np.float32)
        agg_idx[c] = _wrap_idx16(idx_lin)
        agg_dstloc[c] = np.ascontiguousarray(dl_lin.reshape(-1, 128).T)
    plan['agg_idx'] = agg_idx
    plan['agg_dstloc'] = agg_dstloc.astype(bf16)

    # gather blocks: contiguous slot ranges within one src bucket
    blocks = []  # (bucket, slot_start, n_idxs)
    for b in range(nbuck):
        s0 = int(cell_ofs[b * tiles])
        s1 = int(cell_ofs[(b + 1) * tiles])
        s = s0
        while s < s1:
            n = min(ABLK, s1 - s)
            blocks.append((b, s, n))
            s += n
    plan['agg_blocks'] = blocks

    # ---------------- scoring slots (per src-owner core) ------------------
    # segment = bucket(dst); v gathered from local v shard by local_s,
    # u gathered from the full u table by pid_d (bucketed).
    scnt = np.zeros((NC, nbuck), np.int64)
    for c in range(NC):
        m = owner_s == c
        scnt[c] = np.bincount(b_d[m], minlength=nbuck)
    SEG = (np.ceil(scnt.max(axis=0) / P) * P).astype(np.int64)
    seg_ofs = np.concatenate([[0], np.cumsum(SEG)])
    tot_sc = int(seg_ofs[-1])
    totc = tot_sc // P
    totc_pad = ((totc + P - 1) // P) * P
    plan['SEG'] = SEG
    plan['seg_ofs'] = seg_ofs
    plan['tot_sc'] = tot_sc
    plan['totc_pad'] = totc_pad

    sc_v = np.zeros((NC, 128, tot_sc // 16), np.int16)
    sc_u = np.zeros((NC, 128, tot_sc // 16), np.int16)
    sc_core = owner_s.astype(np.int64)          # core of each edge
    sc_slot = np.zeros(E, np.int64)             # slot of edge within core
    for c in range(NC):
        m = owner_s == c
        sl = b_d[m]
        order = np.argsort(sl, kind='stable')
        sl_s = sl[order]
        grp_start = np.searchsorted(sl_s, np.arange(nbuck))
        rank = np.arange(sl_s.shape[0]) - grp_start[sl_s]
        slot = seg_ofs[sl_s] + rank
        eids = np.nonzero(m)[0][order]
        sc_slot[eids] = slot
        vi = np.zeros(tot_sc, np.int16)
        ui = np.zeros(tot_sc, np.int16)
        vi[slot] = local_s[m][order].astype(np.int16)
        pd = pid_d[m][order]
        ui[slot] = (pd - (pd // BUCK) * BUCK).astype(np.int16)
        sc_v[c] = _wrap_idx16(vi)
        sc_u[c] = _wrap_idx16(ui)
    plan['sc_v'] = sc_v
    plan['sc_u'] = sc_u
    plan['sc_core'] = sc_core
    plan['sc_slot'] = sc_slot

    sblocks = []  # (b_dst, slot_start, n_idxs)
    for b in range(nbuck):
        s0 = int(seg_ofs[b])
        s1 = int(seg_ofs[b + 1])
        s = s0
        while s < s1:
            n = min(BLK, s1 - s)
            sblocks.append((b, s, n))
            s += n
    plan['sc_blocks'] = sblocks

    # ---------------- per-core / full node data ---------------------------
    # full z and dinv in cols layout (same for every core): node
    # r = t*128 + p of the FULL padded table at [p, t, :].
    ftiles = nrows // P
    zf = np.zeros((nrows, 16), np.float32)
    df = np.zeros(nrows, np.float32)
    for c in range(NC):
        zf[c * npad:c * npad + npc] = z[c * npc:(c + 1) * npc]
        df[c * npad:c * npad + npc] = dinv[c * npc:(c + 1) * npc]
    plan['z_cols_full'] = np.ascontiguousarray(
        zf.reshape(ftiles, P, 16).transpose(1, 0, 2).reshape(P, ftiles * 16))
    plan['dinv_cols_full'] = np.ascontiguousarray(df.reshape(ftiles, P).T)
    plan['ftiles'] = ftiles

    z_cols = np.zeros((NC, 128, tiles * 16), np.float32)
    dinv_cols = np.zeros((NC, 128, tiles), np.float32)
    for c in range(NC):
        zc = np.zeros((npad, 16), np.float32)
        zc[:npc] = z[c * npc:(c + 1) * npc]
        dc = np.zeros(npad, np.float32)
        dc[:npc] = dinv[c * npc:(c + 1) * npc]
        z_cols[c] = zc.reshape(tiles, P, 16).transpose(1, 0, 2).reshape(P, tiles * 16)
        dinv_cols[c] = dc.reshape(tiles, P).T
    plan['z_cols'] = z_cols
    plan['dinv_cols'] = dinv_cols
    plan['dinv2_cols'] = dinv_cols * dinv_cols
    return plan


def _build(plan, W1np, W2np):
    """Build + compile the SPMD bass program (same program for all cores)."""
    from concourse import bass, bacc, tile, mybir
    from concourse.masks import make_identity

    npad, tiles, nrows, nbuck = plan['npad'], plan['tiles'], plan['nrows'], plan['nbuck']
    ftiles = plan['ftiles']
    tot_agg, tot_sc = plan['tot_agg'], plan['tot_sc']
    totc_pad = plan['totc_pad']
    K, cell_ofs = plan['K'], plan['cell_ofs']
    f32 = mybir.dt.float32
    bf = mybir.dt.bfloat16

    nc = bacc.Bacc("TRN2", target_bir_lowering=False, debug=False, num_devices=NC)

    # ---- I/O ----
    in_zf = nc.dram_tensor("z_cols_full", [128, ftiles * 16], f32, kind="ExternalInput")
    in_df = nc.dram_tensor("dinv_cols_full", [128, ftiles], f32, kind="ExternalInput")
    in_z = nc.dram_tensor("z_cols", [128, tiles * 16], f32, kind="ExternalInput")
    in_dinv = nc.dram_tensor("dinv_cols", [128, tiles], f32, kind="ExternalInput")
    in_dinv2 = nc.dram_tensor("dinv2_cols", [128, tiles], f32, kind="ExternalInput")
    in_w1t = nc.dram_tensor("w1t", [256, 16], f32, kind="ExternalInput")
    in_w2 = nc.dram_tensor("w2", [256, 256], f32, kind="ExternalInput")
    in_aidx = nc.dram_tensor("agg_idx", [128, tot_agg // 16], mybir.dt.int16, kind="ExternalInput")
    in_adl = nc.dram_tensor("agg_dstloc", [128, tot_agg // 128], bf, kind="ExternalInput")
    in_scv = nc.dram_tensor("sc_v", [128, tot_sc // 16], mybir.dt.int16, kind="ExternalInput")
    in_scu = nc.dram_tensor("sc_u", [128, tot_sc // 16], mybir.dt.int16, kind="ExternalInput")
    in_iota = nc.dram_tensor("iota_row", [128, 128], bf, kind="ExternalInput")
    KMAX = plan['KMAX']
    in_iota_rep = nc.dram_tensor("iota_rep", [128, 128 * KMAX], bf, kind="ExternalInput")
    out_val = nc.dram_tensor("out_val", [totc_pad, 128], f32, kind="ExternalOutput")

    with tile.TileContext(nc) as tc:
        with tc.tile_pool(name="res", bufs=1) as res, \
             tc.tile_pool(name="gat", bufs=2) as gat, \
             tc.tile_pool(name="idx", bufs=2) as idxp, \
             tc.tile_pool(name="oh", bufs=4) as ohp, \
             tc.tile_pool(name="sm", bufs=2) as sm, \
             tc.tile_pool(name="ps", bufs=4, space="PSUM") as ps, \
             tc.tile_pool(name="pst", bufs=2, space="PSUM") as pst, \
             tc.tile_pool(name="dram", bufs=1, space="DRAM") as dram:

            # ================= phase 0: constants, t0 table, G ============
            ident = res.tile([128, 128], f32)
            make_identity(nc, ident[:])
            iota = res.tile([128, 128], bf)      # iota along free dim (bf16)
            nc.sync.dma_start(iota[:], in_iota[:])
            iota_rep = res.tile([128, 128 * KMAX], bf)  # value q at col q*KMAX+k
            nc.sync.dma_start(iota_rep[:], in_iota_rep[:])
            ohz = res.tile([128, 128], bf)       # zero lhsT for empty cells
            nc.vector.memset(ohz[:], 0.0)

            dinv_t = res.tile([128, tiles], f32)
            nc.sync.dma_start(dinv_t[:], in_dinv[:])
            dinv2_t = res.tile([128, tiles], f32)
            nc.sync.dma_start(dinv2_t[:], in_dinv2[:])

            # local zt shard (for L1 epilogue)
            zt = res.tile([128, tiles * 16], f32)
            nc.sync.dma_start(zt[:], in_z[:])
            nc.vector.tensor_tensor(
                out=zt[:].rearrange("p (t f) -> p t f", f=16),
                in0=zt[:].rearrange("p (t f) -> p t f", f=16),
                in1=dinv_t[:][:, :, None].to_broadcast([128, tiles, 16]),
                op=mybir.AluOpType.mult)

            # full t0 table, split-bf16, built locally on every core
            # (chunked over column tiles to bound SBUF)
            t0_tab = dram.tile([nrows, BW], bf)
            CH = tiles  # ftiles is a multiple of tiles (= NC * tiles)
            for c0 in range(0, ftiles, CH):
                zf_t = sm.tile([128, CH * 16], f32, tag="zfch")
                nc.sync.dma_start(zf_t[:], in_zf[:, c0 * 16:(c0 + CH) * 16])
                df_t = sm.tile([128, CH], f32, tag="dfch")
                nc.sync.dma_start(df_t[:], in_df[:, c0:c0 + CH])
                nc.vector.tensor_tensor(
                    out=zf_t[:].rearrange("p (t f) -> p t f", f=16),
                    in0=zf_t[:].rearrange("p (t f) -> p t f", f=16),
                    in1=df_t[:][:, :, None].to_broadcast([128, CH, 16]),
                    op=mybir.AluOpType.mult)
                hilo = sm.tile([128, CH * 32], bf, tag="hlch")
                hl_v = hilo[:].rearrange("p (t q f) -> p t q f", q=2, f=16)
                zf_v = zf_t[:].rearrange("p (t f) -> p t f", f=16)
                # hi = bf16(t0); lo = bf16(t0 - f32(hi))
                nc.vector.tensor_copy(hl_v[:, :, 0, :], zf_v)
                nc.vector.tensor_tensor(
                    out=zf_v, in0=zf_v, in1=hl_v[:, :, 0, :],
                    op=mybir.AluOpType.subtract)
                nc.vector.tensor_copy(hl_v[:, :, 1, :], zf_v)
                nc.sync.dma_start(
                    t0_tab[c0 * 128:(c0 + CH) * 128, :]
                        .rearrange("(t p) e -> p t e", p=128)[:, :, 0:32],
                    hilo[:].rearrange("p (t e) -> p t e", e=32))

            # G = (W1 @ W2) @ (W1 @ W2)^T  [16,16]
            w1t_s = res.tile([128, 2 * 16], f32)
            nc.sync.dma_start(w1t_s[:, 0:16], in_w1t[0:128, :])
            nc.sync.dma_start(w1t_s[:, 16:32], in_w1t[128:256, :])
            w2_s = res.tile([128, 2 * 256], f32)
            nc.sync.dma_start(w2_s[:, 0:256], in_w2[0:128, :])
            nc.sync.dma_start(w2_s[:, 256:512], in_w2[128:256, :])
            w12_ps = pst.tile([16, 256], f32, tag="tp", space="PSUM")
            nc.tensor.matmul(w12_ps[:], lhsT=w1t_s[:, 0:16], rhs=w2_s[:, 0:256], start=True, stop=False)
            nc.tensor.matmul(w12_ps[:], lhsT=w1t_s[:, 16:32], rhs=w2_s[:, 256:512], start=False, stop=True)
            w12_s = res.tile([16, 256], f32)
            nc.vector.tensor_copy(w12_s[:], w12_ps[:])
            w12T_s = res.tile([128, 2 * 16], f32)
            for blkk in range(2):
                tp = pst.tile([128, 16], f32, tag="tp", space="PSUM")
                nc.tensor.transpose(tp[:], in_=w12_s[:, blkk * 128:(blkk + 1) * 128], identity=ident[:16, :16])
                nc.vector.tensor_copy(w12T_s[:, blkk * 16:(blkk + 1) * 16], tp[:])
            g_ps = pst.tile([16, 16], f32, tag="tp", space="PSUM")
            nc.tensor.matmul(g_ps[:], lhsT=w12T_s[:, 0:16], rhs=w12T_s[:, 0:16], start=True, stop=False)
            nc.tensor.matmul(g_ps[:], lhsT=w12T_s[:, 16:32], rhs=w12T_s[:, 16:32], start=False, stop=True)
            g_s = res.tile([16, 16], f32)
            nc.vector.tensor_copy(g_s[:], g_ps[:])

            # ================= aggregation layers =========================
            acc = res.tile([128, tiles * 16], f32)
            t1 = res.tile([128, tiles * 16], f32)
            adl_t = res.tile([128, tot_agg // 128], bf)
            nc.sync.dma_start(adl_t[:], in_adl[:])
            rg = [list(range(NC))]

            def agg_layer(table_full, out_sb, scale_t, selfloop_sb):
                """out_sb = scale ⊙ (scatter-sum(table[src]) + selfloop)"""
                nc.vector.memset(acc[:], 0.0)
                blk_tiles = {}

                def get_block(bi):
                    if bi in blk_tiles:
                        return blk_tiles[bi]
                    b, s0, n = plan['agg_blocks'][bi]
                    it = idxp.tile([128, ABLK // 16], mybir.dt.int16, tag="aggidx")
                    nc.sync.dma_start(it[:, :n // 16], in_aidx[:, s0 // 16:(s0 + n) // 16])
                    gt = gat.tile([128, (ABLK // 128) * BW], bf, tag="aggbuf")
                    lo = b * BUCK
                    hi = min(lo + BUCK, nrows)
                    nc.gpsimd.dma_gather(
                        out_ap=gt[:, :(n // 128) * BW].rearrange("p (c f) -> p c f", f=BW),
                        in_ap=table_full[lo:hi, :],
                        idxs_ap=it[:, :n // 16],
                        num_idxs=n, num_idxs_reg=n, elem_size=BW,
                        single_packet=False)
                    blk_tiles[bi] = (gt, s0, n)
                    return blk_tiles[bi]

                bstarts = [b[1] for b in plan['agg_blocks']]
                GB = 16  # cells per psum batch flush
                for b in range(nbuck):
                    for t0b in range(0, tiles, GB):
                        nb = min(GB, tiles - t0b)
                        ptb = ps.tile([128, GB * 16], f32, tag="mm", space="PSUM")
                        for m in range(nb):
                            t = t0b + m
                            kk = int(K[b * tiles + t])
                            pt = ptb[:, m * 16:(m + 1) * 16]
                            if kk == 0:
                                nc.tensor.matmul(
                                    pt, lhsT=ohz[:], rhs=iota[:, 0:16],
                                    start=True, stop=True)
                                continue
                            c0 = int(cell_ofs[b * tiles + t])
                            oh = ohp.tile([128, kk * 128], bf, tag="oh")
                            ohv = oh[:].rearrange("p (q k) -> p q k", k=kk)
                            nc.vector.tensor_tensor(
                                out=ohv,
                                in0=iota_rep[:].rearrange(
                                    "p (q k) -> p q k", k=KMAX)[:, :, 0:kk],
                                in1=adl_t[:, c0 // 128:c0 // 128 + kk][:, None, :]
                                    .to_broadcast([128, 128, kk]),
                                op=mybir.AluOpType.is_equal)
                            for j in range(kk):
                                slot = c0 + j * 128
                                bi = bisect.bisect_right(bstarts, slot) - 1
                                gt, s0, n = get_block(bi)
                                ch = (slot - s0) // 128
                                gv = gt[:].rearrange("p (c f) -> p c f", f=BW)
                                nc.tensor.matmul(
                                    pt, lhsT=ohv[:, :, j], rhs=gv[:, ch, 0:16],
                                    start=(j == 0), stop=False)
                                nc.tensor.matmul(
                                    pt, lhsT=ohv[:, :, j], rhs=gv[:, ch, 16:32],
                                    start=False, stop=(j == kk - 1))
                        nc.vector.tensor_add(
                            out=acc[:, t0b * 16:(t0b + nb) * 16],
                            in0=acc[:, t0b * 16:(t0b + nb) * 16],
                            in1=ptb[:, 0:nb * 16])
                nc.vector.tensor_add(out=out_sb[:], in0=acc[:], in1=selfloop_sb[:])
                nc.vector.tensor_tensor(
                    out=out_sb[:].rearrange("p (t f) -> p t f", f=16),
                    in0=out_sb[:].rearrange("p (t f) -> p t f", f=16),
                    in1=scale_t[:][:, :, None].to_broadcast([128, tiles, 16]),
                    op=mybir.AluOpType.mult)

            def split_pack(src_sb, ncols):
                """f32 cols [128, ncols*16] -> packed hi|lo bf16 [128, ncols*32]"""
                pk = sm.tile([128, ncols * 32], bf, tag="hlch")
                pk_v = pk[:].rearrange("p (t q f) -> p t q f", q=2, f=16)
                rc = sm.tile([128, ncols * 16], f32, tag="zfch")
                rc_v = rc[:].rearrange("p (t f) -> p t f", f=16)
                sv = src_sb[:].rearrange("p (t f) -> p t f", f=16)
                nc.vector.tensor_copy(pk_v[:, :, 0, :], sv)
                nc.vector.tensor_tensor(
                    out=rc_v, in0=sv, in1=pk_v[:, :, 0, :],
                    op=mybir.AluOpType.subtract)
                nc.vector.tensor_copy(pk_v[:, :, 1, :], rc_v)
                return pk

            # ---- L1: t1 = dinv2 ⊙ (agg(t0) + t0local) ----
            agg_layer(t0_tab, t1, dinv2_t, zt)

            # t1 exchange: compact split-bf16 shards -> full padded table
            t1pk = split_pack(t1, tiles)
            t1b = dram.tile([npad, 32], bf)
            nc.sync.dma_start(
                t1b[:].rearrange("(t p) e -> p t e", p=128),
                t1pk[:].rearrange("p (t e) -> p t e", e=32))
            t1_full = dram.tile([nrows, 32], bf)
            nc.gpsimd.collective_compute(
                "AllGather", mybir.AluOpType.bypass,
                ins=[t1b.opt()], outs=[t1_full.opt()], replica_groups=rg)
            t1_tab = dram.tile([nrows, BW], bf)
            half = (nrows // 2) // 128 * 128
            for r0, r1 in ((0, half), (half, nrows)):
                nc.sync.dma_start(
                    t1_tab[r0:r1, 0:32], t1_full[r0:r1, :])

            # ---- L2: u = dinv ⊙ (agg(t1) + t1) ----
            u_sb = res.tile([128, tiles * 16], f32)
            agg_layer(t1_tab, u_sb, dinv_t, t1)

            # u exchange first: the collective runs while v is computed
            ub = dram.tile([npad, 16], f32)
            nc.sync.dma_start(
                ub[:].rearrange("(t p) e -> p t e", p=128),
                u_sb[:].rearrange("p (t f) -> p t f", f=16))
            u_full = dram.tile([nrows, 16], f32)
            nc.gpsimd.collective_compute(
                "AllGather", mybir.AluOpType.bypass,
                ins=[ub.opt()], outs=[u_full.opt()], replica_groups=rg)

            # v = u @ G per tile (overlaps the u AllGather)
            v_sb = res.tile([128, tiles * 16], f32)
            for t in range(tiles):
                tp = pst.tile([16, 128], f32, tag="tp", space="PSUM")
                nc.tensor.transpose(tp[:], in_=u_sb[:, t * 16:(t + 1) * 16], identity=ident[:])
                uT = sm.tile([16, 128], f32, tag="uTs")
                nc.vector.tensor_copy(uT[:], tp[:])
                vp = ps.tile([128, 16], f32, tag="mm", space="PSUM")
                nc.tensor.matmul(vp[:], lhsT=uT[:], rhs=g_s[:], start=True, stop=True)
                nc.vector.tensor_copy(v_sb[:, t * 16:(t + 1) * 16], vp[:])

            # v shard table (local only, f32 rows)
            v_tab = dram.tile([npad, FW], f32)
            nc.sync.dma_start(
                v_tab[:].rearrange("(t p) e -> p t e", p=128)[:, :, 0:16],
                v_sb[:].rearrange("p (t f) -> p t f", f=16))

            u_tab = dram.tile([nrows, FW], f32)
            for r0, r1 in ((0, half), (half, nrows)):
                nc.sync.dma_start(
                    u_tab[r0:r1, 0:16], u_full[r0:r1, :])

            # ================= scoring ====================================
            val = res.tile([128, totc_pad], f32)
            nc.vector.memset(val[:], 0.0)
            for (b, s0, n) in plan['sc_blocks']:
                itv = idxp.tile([128, BLK // 16], mybir.dt.int16, tag="scidxv")
                nc.sync.dma_start(itv[:, :n // 16], in_scv[:, s0 // 16:(s0 + n) // 16])
                itu = idxp.tile([128, BLK // 16], mybir.dt.int16, tag="scidxu")
                nc.sync.dma_start(itu[:, :n // 16], in_scu[:, s0 // 16:(s0 + n) // 16])
                gv = gat.tile([128, (BLK // 128) * FW], f32, tag="aggbuf")
                gu = gat.tile([128, (BLK // 128) * FW], f32, tag="scubuf")
                nc.gpsimd.dma_gather(
                    out_ap=gv[:, :(n // 128) * FW].rearrange("p (c f) -> p c f", f=FW),
                    in_ap=v_tab[:, :],
                    idxs_ap=itv[:, :n // 16],
                    num_idxs=n, num_idxs_reg=n, elem_size=FW, single_packet=False)
                lo = b * BUCK
                nc.gpsimd.dma_gather(
                    out_ap=gu[:, :(n // 128) * FW].rearrange("p (c f) -> p c f", f=FW),
                    in_ap=u_tab[lo:min(lo + BUCK, nrows), :],
                    idxs_ap=itu[:, :n // 16],
                    num_idxs=n, num_idxs_reg=n, elem_size=FW, single_packet=False)
                nch = n // 128
                prod = sm.tile([128, (BLK // 128) * 16], f32, tag="prod")
                nc.vector.tensor_tensor(
                    out=prod[:, :nch * 16].rearrange("p (c f) -> p c f", f=16),
                    in0=gv[:].rearrange("p (c f) -> p c f", f=FW)[:, 0:nch, 0:16],
                    in1=gu[:].rearrange("p (c f) -> p c f", f=FW)[:, 0:nch, 0:16],
                    op=mybir.AluOpType.mult)
                nc.vector.reduce_sum(
                    out=val[:, s0 // 128:s0 // 128 + nch],
                    in_=prod[:, :nch * 16].rearrange("p (c f) -> p c f", f=16),
                    axis=mybir.AxisListType.X)

            # sigmoid + transpose + out
            for g in range(totc_pad // 128):
                sg = sm.tile([128, 128], f32, tag="sig")
                nc.scalar.activation(sg[:], val[:, g * 128:(g + 1) * 128],
                                     mybir.ActivationFunctionType.Sigmoid)
                tp = pst.tile([128, 128], f32, tag="tp", space="PSUM")
                nc.tensor.transpose(tp[:], in_=sg[:], identity=ident[:])
                so = sm.tile([128, 128], f32, tag="sigT")
                nc.vector.tensor_copy(so[:], tp[:])
                nc.sync.dma_start(out_val[g * 128:(g + 1) * 128, :], so[:])

    nc.compile()
    return nc


_CACHE = {}


def kernel(z, edge_index, W1, b1, W2, b2):
    z = np.asarray(z, np.float32)
    edge_index = np.asarray(edge_index)
    W1 = np.asarray(W1, np.float32)
    W2 = np.asarray(W2, np.float32)
    b1 = np.asarray(b1, np.float32)
    b2 = np.asarray(b2, np.float32)
    if np.any(b1 != 0) or np.any(b2 != 0):
        return _host_reference(z, edge_index, W1, b1, W2, b2)

    from concourse import bass_utils

    plan = _plan(z, edge_index)
    key = (z.shape, edge_index.shape, plan['tot_agg'], plan['tot_sc'],
           tuple(plan['K'].tolist()), tuple(plan['SEG'].tolist()))
    if key not in _CACHE:
        _CACHE.clear()
        _CACHE[key] = _build(plan, W1, W2)
    nc = _CACHE[key]

    w1t = np.ascontiguousarray(W1.T)
    iota_row = np.tile(np.arange(128, dtype=np.float32), (128, 1)).astype(bf16)
    KMAX = plan['KMAX']
    iota_rep = np.tile(np.repeat(np.arange(128, dtype=np.float32), KMAX), (128, 1)).astype(bf16)
    in_maps = []
    for c in range(NC):
        in_maps.append({
            "z_cols_full": plan['z_cols_full'],
            "dinv_cols_full": plan['dinv_cols_full'],
            "z_cols": plan['z_cols'][c],
            "dinv_cols": plan['dinv_cols'][c],
            "dinv2_cols": plan['dinv2_cols'][c],
            "w1t": w1t, "w2": W2,
            "agg_idx": plan['agg_idx'][c],
            "agg_dstloc": plan['agg_dstloc'][c],
            "sc_v": plan['sc_v'][c],
            "sc_u": plan['sc_u'][c],
            "iota_row": iota_row,
            "iota_rep": iota_rep,
        })
    res = bass_utils.run_bass_kernel_spmd(nc, in_maps, core_ids=list(range(NC)))
    kernel._last = (nc, in_maps, plan)

    E = plan['E']
    flat = np.stack([res.results[c]["out_val"].reshape(-1) for c in range(NC)])
    out = flat[plan['sc_core'], plan['sc_slot']].astype(np.float32)
    return out


# revision 15
# speedup vs baseline: 1.0560x; 1.0560x over previous
"""GCN link-decoder kernel for 8 TRN2 NeuronCores (v2).

Math: both GCNConv layers are linear (no activation), so with
P = D^-1/2 (A+I) D^-1/2 the network output is
    value_e = sigmoid( h2[src_e] . h2[dst_e] ),  h2 = P^2 z W1 W2  (b1=b2=0)
which reduces to 16-dim aggregations:
    t0 = dinv * z;  agg1 = sum_{e into d} t0[src_e];  t1 = dinv^2*(agg1 + t0)
    agg2 = sum t1[src_e];  u = dinv*(agg2 + t1);  v = u @ G,  G = (W1W2)(W1W2)^T
    value_e = v[src_e] . u[dst_e]

v2 structure (vs v1): aggregation tables are stored as split-bf16 pairs
(hi|lo) so the scatter one-hot matmuls and their is_equal builds run at
bf16 DVE/PE rates while retaining ~fp32 accuracy; the t0 table is built
locally on every core from the full z input (no first AllGather); the t1
and u exchanges ship compact [npad,32|16] shards (3.2MB) instead of padded
256B-row tables (25.7MB); scoring runs on the src-owner core so v[src] is
gathered from the local v shard (never exchanged) and only u is
AllGathered; scoring math stays fp32 (the sigmoid tail needs ~2e-2
absolute logit accuracy).
"""
import sys
import os
import bisect
sys.path.insert(0, '/opt/trn_rl_repo')
import numpy as np
import ml_dtypes

NC = 8          # cores
P = 128         # partitions / chunk size
FW = 64         # f32 table row width (256B granule)
BW = 128        # bf16 table row width (256B granule)
BUCK = 32768    # int16 index bucket size (table rows per bucket)
BLK = 8192      # scoring gather idxs per dma_gather instruction
ABLK = 8192     # aggregation gather idxs per dma_gather instruction

bf16 = ml_dtypes.bfloat16


def _wrap_idx16(arr: np.ndarray) -> np.ndarray:
    """Linear int16 slot-index array (len % 128 == 0) -> [128, len/16] SWDGE
    wrapped layout (slot k at partition k%16, col k//16; 16-row pattern
    replicated to 128 partitions)."""
    n = arr.shape[0]
    t16 = arr.reshape(n // 16, 16).T
    return np.ascontiguousarray(np.tile(t16, (8, 1)))


def _host_reference(z, edge_index, W1, b1, W2, b2):
    """Numpy fallback (used only when b1/b2 are nonzero)."""
    N = z.shape[0]
    src, dst = edge_index[0], edge_index[1]
    deg = (np.bincount(dst, minlength=N) + 1.0).astype(np.float64)
    dinv = (1.0 / np.sqrt(deg)).astype(np.float32)

    def conv(x, W, b):
        h = x @ W
        out = np.zeros_like(h)
        np.add.at(out, dst, h[src] * (dinv[src] * dinv[dst])[:, None])
        out += h * (dinv * dinv)[:, None]
        return out + b

    h = conv(z, W1, b1)
    h = conv(h, W2, b2)
    val = np.einsum('ef,ef->e', h[src], h[dst]).astype(np.float64)
    return (1.0 / (1.0 + np.exp(-val))).astype(np.float32)


def _plan(z, edge_index):
    """Host-side layout planning: shard nodes/edges, build slot arrays."""
    N = z.shape[0]
    E = edge_index.shape[1]
    assert N % NC == 0 and E % NC == 0
    npc = N // NC                      # real nodes per core
    npad = ((npc + P - 1) // P) * P    # padded nodes per core
    tiles = npad // P
    nrows = NC * npad                  # table rows
    nbuck = (nrows + BUCK - 1) // BUCK

    src = edge_index[0].astype(np.int64)
    dst = edge_index[1].astype(np.int64)
    deg = np.bincount(dst, minlength=N).astype(np.float64) + 1.0
    dinv = (1.0 / np.sqrt(deg)).astype(np.float32)

    owner_s, local_s = src // npc, src % npc
    owner_d, local_d = dst // npc, dst % npc
    pid_s = (owner_s * npad + local_s).astype(np.int64)
    pid_d = (owner_d * npad + local_d).astype(np.int64)
    b_s = (pid_s // BUCK).astype(np.int64)
    b_d = (pid_d // BUCK).astype(np.int64)

    plan = {
        'N': N, 'E': E, 'npc': npc, 'npad': npad, 'tiles': tiles,
        'nrows': nrows, 'nbuck': nbuck, 'dinv': dinv,
    }

    # ---------------- aggregation slots (per dst-owner core) --------------
    # cell = (bucket(src), dst_tile); bucket-major order.
    t_d = local_d // P                     # dst tile within owner
    cell = b_s * tiles + t_d               # cell id within owner core
    ncell = nbuck * tiles
    counts = np.zeros((NC, ncell), np.int64)
    for c in range(NC):
        m = owner_d == c
        counts[c] = np.bincount(cell[m], minlength=ncell)
    K = np.maximum(np.ceil(counts.max(axis=0) / P).astype(np.int64), 0)
    cell_slots = K * P
    cell_ofs = np.concatenate([[0], np.cumsum(cell_slots)])
    tot_agg = int(cell_ofs[-1])
    plan['K'] = K
    plan['cell_ofs'] = cell_ofs
    plan['tot_agg'] = tot_agg
    plan['KMAX'] = max(int(K.max()), 1)

    agg_idx = np.zeros((NC, 128, tot_agg // 16), np.int16)
    agg_dstloc = np.full((NC, 128, tot_agg // 128), -1.0, np.float32)
    for c in range(NC):
        m = owner_d == c
        cl = cell[m]
        order = np.argsort(cl, kind='stable')
        cl_s = cl[order]
        grp_start = np.searchsorted(cl_s, np.arange(ncell))
        rank = np.arange(cl_s.shape[0]) - grp_start[cl_s]
        slot = cell_ofs[cl_s] + rank
        idx_lin = np.zeros(tot_agg, np.int16)
        dl_lin = np.full(tot_agg, -1.0, np.float32)
        ps = pid_s[m][order]
        idx_lin[slot] = (ps - (ps // BUCK) * BUCK).astype(np.int16)
        dl_lin[slot] = (local_d[m][order] % P).astype(np.float32)
        agg_idx[c] = _wrap_idx16(idx_lin)
        agg_dstloc[c] = np.ascontiguousarray(dl_lin.reshape(-1, 128).T)
    plan['agg_idx'] = agg_idx
    plan['agg_dstloc'] = agg_dstloc.astype(bf16)

    # gather blocks: contiguous slot ranges within one src bucket
    blocks = []  # (bucket, slot_start, n_idxs)
    for b in range(nbuck):
        s0 = int(cell_ofs[b * tiles])
        s1 = int(cell_ofs[(b + 1) * tiles])
        s = s0
        while s < s1:
            n = min(ABLK, s1 - s)
            blocks.append((b, s, n))
            s += n
    plan['agg_blocks'] = blocks

    # ---------------- scoring slots (per src-owner core) ------------------
    # segment = bucket(dst); v gathered from local v shard by local_s,
    # u gathered from the full u table by pid_d (bucketed).
    ph_d = (pid_d % 4).astype(np.int64)
    scnt = np.zeros((NC, 4), np.int64)
    for c in range(NC):
        m = owner_s == c
        scnt[c] = np.bincount(ph_d[m], minlength=4)
    SEG = (np.ceil(scnt.max(axis=0) / P) * P).astype(np.int64)
    seg_ofs = np.concatenate([[0], np.cumsum(SEG)])
    tot_sc = int(seg_ofs[-1])
    totc = tot_sc // P
    totc_pad = ((totc + P - 1) // P) * P
    plan['SEG'] = SEG
    plan['seg_ofs'] = seg_ofs
    plan['tot_sc'] = tot_sc
    plan['totc_pad'] = totc_pad

    sc_v = np.zeros((NC, 128, tot_sc // 16), np.int16)
    sc_u = np.zeros((NC, 128, tot_sc // 16), np.int16)
    sc_core = owner_s.astype(np.int64)          # core of each edge
    sc_slot = np.zeros(E, np.int64)             # slot of edge within core
    for c in range(NC):
        m = owner_s == c
        sl = ph_d[m]
        order = np.argsort(sl, kind='stable')
        sl_s = sl[order]
        grp_start = np.searchsorted(sl_s, np.arange(4))
        rank = np.arange(sl_s.shape[0]) - grp_start[sl_s]
        slot = seg_ofs[sl_s] + rank
        eids = np.nonzero(m)[0][order]
        sc_slot[eids] = slot
        vi = np.zeros(tot_sc, np.int16)
        ui = np.zeros(tot_sc, np.int16)
        vi[slot] = local_s[m][order].astype(np.int16)
        ui[slot] = (pid_d[m][order] // 4).astype(np.int16)
        sc_v[c] = _wrap_idx16(vi)
        sc_u[c] = _wrap_idx16(ui)
    plan['sc_v'] = sc_v
    plan['sc_u'] = sc_u
    plan['sc_core'] = sc_core
    plan['sc_slot'] = sc_slot

    sblocks = []  # (dst_phase, slot_start, n_idxs)
    for b in range(4):
        s0 = int(seg_ofs[b])
        s1 = int(seg_ofs[b + 1])
        s = s0
        while s < s1:
            n = min(BLK, s1 - s)
            sblocks.append((b, s, n))
            s += n
    plan['sc_blocks'] = sblocks

    # ---------------- per-core / full node data ---------------------------
    # full z and dinv in cols layout (same for every core): node
    # r = t*128 + p of the FULL padded table at [p, t, :].
    ftiles = nrows // P
    zf = np.zeros((nrows, 16), np.float32)
    df = np.zeros(nrows, np.float32)
    for c in range(NC):
        zf[c * npad:c * npad + npc] = z[c * npc:(c + 1) * npc]
        df[c * npad:c * npad + npc] = dinv[c * npc:(c + 1) * npc]
    plan['z_cols_full'] = np.ascontiguousarray(
        zf.reshape(ftiles, P, 16).transpose(1, 0, 2).reshape(P, ftiles * 16))
    plan['dinv_cols_full'] = np.ascontiguousarray(df.reshape(ftiles, P).T)
    plan['ftiles'] = ftiles

    z_cols = np.zeros((NC, 128, tiles * 16), np.float32)
    dinv_cols = np.zeros((NC, 128, tiles), np.float32)
    for c in range(NC):
        zc = np.zeros((npad, 16), np.float32)
        zc[:npc] = z[c * npc:(c + 1) * npc]
        dc = np.zeros(npad, np.float32)
        dc[:npc] = dinv[c * npc:(c + 1) * npc]
        z_cols[c] = zc.reshape(tiles, P, 16).transpose(1, 0, 2).reshape(P, tiles * 16)
        dinv_cols[c] = dc.reshape(tiles, P).T
    plan['z_cols'] = z_cols
    plan['dinv_cols'] = dinv_cols
    plan['dinv2_cols'] = dinv_cols * dinv_cols
    return plan


def _build(plan, W1np, W2np):
    """Build + compile the SPMD bass program (same program for all cores)."""
    from concourse import bass, bacc, tile, mybir
    from concourse.masks import make_identity

    npad, tiles, nrows, nbuck = plan['npad'], plan['tiles'], plan['nrows'], plan['nbuck']
    ftiles = plan['ftiles']
    tot_agg, tot_sc = plan['tot_agg'], plan['tot_sc']
    totc_pad = plan['totc_pad']
    K, cell_ofs = plan['K'], plan['cell_ofs']
    f32 = mybir.dt.float32
    bf = mybir.dt.bfloat16

    nc = bacc.Bacc("TRN2", target_bir_lowering=False, debug=False, num_devices=NC)

    # ---- I/O ----
    in_zf = nc.dram_tensor("z_cols_full", [128, ftiles * 16], f32, kind="ExternalInput")
    in_df = nc.dram_tensor("dinv_cols_full", [128, ftiles], f32, kind="ExternalInput")
    in_z = nc.dram_tensor("z_cols", [128, tiles * 16], f32, kind="ExternalInput")
    in_dinv = nc.dram_tensor("dinv_cols", [128, tiles], f32, kind="ExternalInput")
    in_dinv2 = nc.dram_tensor("dinv2_cols", [128, tiles], f32, kind="ExternalInput")
    in_w1t = nc.dram_tensor("w1t", [256, 16], f32, kind="ExternalInput")
    in_w2 = nc.dram_tensor("w2", [256, 256], f32, kind="ExternalInput")
    in_aidx = nc.dram_tensor("agg_idx", [128, tot_agg // 16], mybir.dt.int16, kind="ExternalInput")
    in_adl = nc.dram_tensor("agg_dstloc", [128, tot_agg // 128], bf, kind="ExternalInput")
    in_scv = nc.dram_tensor("sc_v", [128, tot_sc // 16], mybir.dt.int16, kind="ExternalInput")
    in_scu = nc.dram_tensor("sc_u", [128, tot_sc // 16], mybir.dt.int16, kind="ExternalInput")
    in_iota = nc.dram_tensor("iota_row", [128, 128], bf, kind="ExternalInput")
    KMAX = plan['KMAX']
    in_iota_rep = nc.dram_tensor("iota_rep", [128, 128 * KMAX], bf, kind="ExternalInput")
    out_val = nc.dram_tensor("out_val", [totc_pad, 128], f32, kind="ExternalOutput")

    with tile.TileContext(nc) as tc:
        with tc.tile_pool(name="res", bufs=1) as res, \
             tc.tile_pool(name="gat", bufs=2) as gat, \
             tc.tile_pool(name="idx", bufs=2) as idxp, \
             tc.tile_pool(name="oh", bufs=4) as ohp, \
             tc.tile_pool(name="sm", bufs=2) as sm, \
             tc.tile_pool(name="ps", bufs=4, space="PSUM") as ps, \
             tc.tile_pool(name="pst", bufs=2, space="PSUM") as pst, \
             tc.tile_pool(name="dram", bufs=1, space="DRAM") as dram:

            # ================= phase 0: constants, t0 table, G ============
            ident = res.tile([128, 128], f32)
            make_identity(nc, ident[:])
            iota = res.tile([128, 128], bf)      # iota along free dim (bf16)
            nc.sync.dma_start(iota[:], in_iota[:])
            iota_rep = res.tile([128, 128 * KMAX], bf)  # value q at col q*KMAX+k
            nc.sync.dma_start(iota_rep[:], in_iota_rep[:])
            ohz = res.tile([128, 128], bf)       # zero lhsT for empty cells
            nc.vector.memset(ohz[:], 0.0)

            dinv_t = res.tile([128, tiles], f32)
            nc.sync.dma_start(dinv_t[:], in_dinv[:])
            dinv2_t = res.tile([128, tiles], f32)
            nc.sync.dma_start(dinv2_t[:], in_dinv2[:])

            # local zt shard (for L1 epilogue)
            zt = res.tile([128, tiles * 16], f32)
            nc.sync.dma_start(zt[:], in_z[:])
            nc.vector.tensor_tensor(
                out=zt[:].rearrange("p (t f) -> p t f", f=16),
                in0=zt[:].rearrange("p (t f) -> p t f", f=16),
                in1=dinv_t[:][:, :, None].to_broadcast([128, tiles, 16]),
                op=mybir.AluOpType.mult)

            # full t0 table, split-bf16, built locally on every core
            # (chunked over column tiles to bound SBUF)
            t0_tab = dram.tile([nrows, BW], bf)
            CH = tiles  # ftiles is a multiple of tiles (= NC * tiles)
            for c0 in range(0, ftiles, CH):
                zf_t = sm.tile([128, CH * 16], f32, tag="zfch")
                nc.sync.dma_start(zf_t[:], in_zf[:, c0 * 16:(c0 + CH) * 16])
                df_t = sm.tile([128, CH], f32, tag="dfch")
                nc.sync.dma_start(df_t[:], in_df[:, c0:c0 + CH])
                nc.vector.tensor_tensor(
                    out=zf_t[:].rearrange("p (t f) -> p t f", f=16),
                    in0=zf_t[:].rearrange("p (t f) -> p t f", f=16),
                    in1=df_t[:][:, :, None].to_broadcast([128, CH, 16]),
                    op=mybir.AluOpType.mult)
                hilo = sm.tile([128, CH * 32], bf, tag="hlch")
                hl_v = hilo[:].rearrange("p (t q f) -> p t q f", q=2, f=16)
                zf_v = zf_t[:].rearrange("p (t f) -> p t f", f=16)
                # hi = bf16(t0); lo = bf16(t0 - f32(hi))
                nc.vector.tensor_copy(hl_v[:, :, 0, :], zf_v)
                nc.vector.tensor_tensor(
                    out=zf_v, in0=zf_v, in1=hl_v[:, :, 0, :],
                    op=mybir.AluOpType.subtract)
                nc.vector.tensor_copy(hl_v[:, :, 1, :], zf_v)
                nc.sync.dma_start(
                    t0_tab[c0 * 128:(c0 + CH) * 128, :]
                        .rearrange("(t p) e -> p t e", p=128)[:, :, 0:32],
                    hilo[:].rearrange("p (t e) -> p t e", e=32))

            # G = (W1 @ W2) @ (W1 @ W2)^T  [16,16]
            w1t_s = res.tile([128, 2 * 16], f32)
            nc.sync.dma_start(w1t_s[:, 0:16], in_w1t[0:128, :])
            nc.sync.dma_start(w1t_s[:, 16:32], in_w1t[128:256, :])
            w2_s = res.tile([128, 2 * 256], f32)
            nc.sync.dma_start(w2_s[:, 0:256], in_w2[0:128, :])
            nc.sync.dma_start(w2_s[:, 256:512], in_w2[128:256, :])
            w12_ps = pst.tile([16, 256], f32, tag="tp", space="PSUM")
            nc.tensor.matmul(w12_ps[:], lhsT=w1t_s[:, 0:16], rhs=w2_s[:, 0:256], start=True, stop=False)
            nc.tensor.matmul(w12_ps[:], lhsT=w1t_s[:, 16:32], rhs=w2_s[:, 256:512], start=False, stop=True)
            w12_s = res.tile([16, 256], f32)
            nc.vector.tensor_copy(w12_s[:], w12_ps[:])
            w12T_s = res.tile([128, 2 * 16], f32)
            for blkk in range(2):
                tp = pst.tile([128, 16], f32, tag="tp", space="PSUM")
                nc.tensor.transpose(tp[:], in_=w12_s[:, blkk * 128:(blkk + 1) * 128], identity=ident[:16, :16])
                nc.vector.tensor_copy(w12T_s[:, blkk * 16:(blkk + 1) * 16], tp[:])
            g_ps = pst.tile([16, 16], f32, tag="tp", space="PSUM")
            nc.tensor.matmul(g_ps[:], lhsT=w12T_s[:, 0:16], rhs=w12T_s[:, 0:16], start=True, stop=False)
            nc.tensor.matmul(g_ps[:], lhsT=w12T_s[:, 16:32], rhs=w12T_s[:, 16:32], start=False, stop=True)
            g_s = res.tile([16, 16], f32)
            nc.vector.tensor_copy(g_s[:], g_ps[:])

            # ================= aggregation layers =========================
            acc = res.tile([128, tiles * 16], f32)
            t1 = res.tile([128, tiles * 16], f32)
            adl_t = res.tile([128, tot_agg // 128], bf)
            nc.sync.dma_start(adl_t[:], in_adl[:])
            rg = [list(range(NC))]

            def agg_layer(table_full, out_sb, scale_t, selfloop_sb):
                """out_sb = scale ⊙ (scatter-sum(table[src]) + selfloop)"""
                nc.vector.memset(acc[:], 0.0)
                blk_tiles = {}

                def get_block(bi):
                    if bi in blk_tiles:
                        return blk_tiles[bi]
                    b, s0, n = plan['agg_blocks'][bi]
                    it = idxp.tile([128, ABLK // 16], mybir.dt.int16, tag="aggidx")
                    nc.sync.dma_start(it[:, :n // 16], in_aidx[:, s0 // 16:(s0 + n) // 16])
                    gt = gat.tile([128, (ABLK // 128) * BW], bf, tag="aggbuf")
                    lo = b * BUCK
                    hi = min(lo + BUCK, nrows)
                    nc.gpsimd.dma_gather(
                        out_ap=gt[:, :(n // 128) * BW].rearrange("p (c f) -> p c f", f=BW),
                        in_ap=table_full[lo:hi, :],
                        idxs_ap=it[:, :n // 16],
                        num_idxs=n, num_idxs_reg=n, elem_size=BW,
                        single_packet=False)
                    blk_tiles[bi] = (gt, s0, n)
                    return blk_tiles[bi]

                bstarts = [b[1] for b in plan['agg_blocks']]
                GB = 16  # cells per psum batch flush
                for b in range(nbuck):
                    for t0b in range(0, tiles, GB):
                        nb = min(GB, tiles - t0b)
                        ptb = ps.tile([128, GB * 16], f32, tag="mm", space="PSUM")
                        for m in range(nb):
                            t = t0b + m
                            kk = int(K[b * tiles + t])
                            pt = ptb[:, m * 16:(m + 1) * 16]
                            if kk == 0:
                                nc.tensor.matmul(
                                    pt, lhsT=ohz[:], rhs=iota[:, 0:16],
                                    start=True, stop=True)
                                continue
                            c0 = int(cell_ofs[b * tiles + t])
                            oh = ohp.tile([128, kk * 128], bf, tag="oh")
                            ohv = oh[:].rearrange("p (q k) -> p q k", k=kk)
                            nc.vector.tensor_tensor(
                                out=ohv,
                                in0=iota_rep[:].rearrange(
                                    "p (q k) -> p q k", k=KMAX)[:, :, 0:kk],
                                in1=adl_t[:, c0 // 128:c0 // 128 + kk][:, None, :]
                                    .to_broadcast([128, 128, kk]),
                                op=mybir.AluOpType.is_equal)
                            for j in range(kk):
                                slot = c0 + j * 128
                                bi = bisect.bisect_right(bstarts, slot) - 1
                                gt, s0, n = get_block(bi)
                                ch = (slot - s0) // 128
                                gv = gt[:].rearrange("p (c f) -> p c f", f=BW)
                                nc.tensor.matmul(
                                    pt, lhsT=ohv[:, :, j], rhs=gv[:, ch, 0:16],
                                    start=(j == 0), stop=False)
                                nc.tensor.matmul(
                                    pt, lhsT=ohv[:, :, j], rhs=gv[:, ch, 16:32],
                                    start=False, stop=(j == kk - 1))
                        nc.vector.tensor_add(
                            out=acc[:, t0b * 16:(t0b + nb) * 16],
                            in0=acc[:, t0b * 16:(t0b + nb) * 16],
                            in1=ptb[:, 0:nb * 16])
                nc.vector.tensor_add(out=out_sb[:], in0=acc[:], in1=selfloop_sb[:])
                nc.vector.tensor_tensor(
                    out=out_sb[:].rearrange("p (t f) -> p t f", f=16),
                    in0=out_sb[:].rearrange("p (t f) -> p t f", f=16),
                    in1=scale_t[:][:, :, None].to_broadcast([128, tiles, 16]),
                    op=mybir.AluOpType.mult)

            def split_pack(src_sb, ncols):
                """f32 cols [128, ncols*16] -> packed hi|lo bf16 [128, ncols*32]"""
                pk = sm.tile([128, ncols * 32], bf, tag="hlch")
                pk_v = pk[:].rearrange("p (t q f) -> p t q f", q=2, f=16)
                rc = sm.tile([128, ncols * 16], f32, tag="zfch")
                rc_v = rc[:].rearrange("p (t f) -> p t f", f=16)
                sv = src_sb[:].rearrange("p (t f) -> p t f", f=16)
                nc.vector.tensor_copy(pk_v[:, :, 0, :], sv)
                nc.vector.tensor_tensor(
                    out=rc_v, in0=sv, in1=pk_v[:, :, 0, :],
                    op=mybir.AluOpType.subtract)
                nc.vector.tensor_copy(pk_v[:, :, 1, :], rc_v)
                return pk

            # ---- L1: t1 = dinv2 ⊙ (agg(t0) + t0local) ----
            agg_layer(t0_tab, t1, dinv2_t, zt)

            # t1 exchange: compact split-bf16 shards -> full padded table
            t1pk = split_pack(t1, tiles)
            t1b = dram.tile([npad, 32], bf)
            nc.sync.dma_start(
                t1b[:].rearrange("(t p) e -> p t e", p=128),
                t1pk[:].rearrange("p (t e) -> p t e", e=32))
            t1_full = dram.tile([nrows, 32], bf)
            nc.gpsimd.collective_compute(
                "AllGather", mybir.AluOpType.bypass,
                ins=[t1b.opt()], outs=[t1_full.opt()], replica_groups=rg)
            t1_tab = dram.tile([nrows, BW], bf)
            half = (nrows // 2) // 128 * 128
            for r0, r1 in ((0, half), (half, nrows)):
                nc.sync.dma_start(
                    t1_tab[r0:r1, 0:32], t1_full[r0:r1, :])

            # ---- L2: u = dinv ⊙ (agg(t1) + t1) ----
            u_sb = res.tile([128, tiles * 16], f32)
            agg_layer(t1_tab, u_sb, dinv_t, t1)

            # u exchange first: the collective runs while v is computed
            ub = dram.tile([npad, 16], f32)
            nc.sync.dma_start(
                ub[:].rearrange("(t p) e -> p t e", p=128),
                u_sb[:].rearrange("p (t f) -> p t f", f=16))
            u_full = dram.tile([nrows, 16], f32)
            nc.gpsimd.collective_compute(
                "AllGather", mybir.AluOpType.bypass,
                ins=[ub.opt()], outs=[u_full.opt()], replica_groups=rg)

            # v = u @ G per tile (overlaps the u AllGather)
            v_sb = res.tile([128, tiles * 16], f32)
            for t in range(tiles):
                tp = pst.tile([16, 128], f32, tag="tp", space="PSUM")
                nc.tensor.transpose(tp[:], in_=u_sb[:, t * 16:(t + 1) * 16], identity=ident[:])
                uT = sm.tile([16, 128], f32, tag="uTs")
                nc.vector.tensor_copy(uT[:], tp[:])
                vp = ps.tile([128, 16], f32, tag="mm", space="PSUM")
                nc.tensor.matmul(vp[:], lhsT=uT[:], rhs=g_s[:], start=True, stop=True)
                nc.vector.tensor_copy(v_sb[:, t * 16:(t + 1) * 16], vp[:])

            # v shard table (local only, f32 rows)
            v_tab = dram.tile([npad, FW], f32)
            nc.sync.dma_start(
                v_tab[:].rearrange("(t p) e -> p t e", p=128)[:, :, 0:16],
                v_sb[:].rearrange("p (t f) -> p t f", f=16))


            # ================= scoring ====================================
            val = res.tile([128, totc_pad], f32)
            nc.vector.memset(val[:], 0.0)
            for (b, s0, n) in plan['sc_blocks']:
                itv = idxp.tile([128, BLK // 16], mybir.dt.int16, tag="scidxv")
                nc.sync.dma_start(itv[:, :n // 16], in_scv[:, s0 // 16:(s0 + n) // 16])
                itu = idxp.tile([128, BLK // 16], mybir.dt.int16, tag="scidxu")
                nc.sync.dma_start(itu[:, :n // 16], in_scu[:, s0 // 16:(s0 + n) // 16])
                gv = gat.tile([128, (BLK // 128) * FW], f32, tag="aggbuf")
                gu = gat.tile([128, (BLK // 128) * FW], f32, tag="scubuf")
                nc.gpsimd.dma_gather(
                    out_ap=gv[:, :(n // 128) * FW].rearrange("p (c f) -> p c f", f=FW),
                    in_ap=v_tab[:, :],
                    idxs_ap=itv[:, :n // 16],
                    num_idxs=n, num_idxs_reg=n, elem_size=FW, single_packet=False)
                nc.gpsimd.dma_gather(
                    out_ap=gu[:, :(n // 128) * FW].rearrange("p (c f) -> p c f", f=FW),
                    in_ap=u_full[:].rearrange("(a b) e -> a (b e)", b=4),
                    idxs_ap=itu[:, :n // 16],
                    num_idxs=n, num_idxs_reg=n, elem_size=FW, single_packet=False)
                nch = n // 128
                prod = sm.tile([128, (BLK // 128) * 16], f32, tag="prod")
                nc.vector.tensor_tensor(
                    out=prod[:, :nch * 16].rearrange("p (c f) -> p c f", f=16),
                    in0=gv[:].rearrange("p (c f) -> p c f", f=FW)[:, 0:nch, 0:16],
                    in1=gu[:].rearrange("p (c f) -> p c f", f=FW)[:, 0:nch, b * 16:b * 16 + 16],
                    op=mybir.AluOpType.mult)
                nc.vector.reduce_sum(
                    out=val[:, s0 // 128:s0 // 128 + nch],
                    in_=prod[:, :nch * 16].rearrange("p (c f) -> p c f", f=16),
                    axis=mybir.AxisListType.X)

            # sigmoid + transpose + out
            for g in range(totc_pad // 128):
                sg = sm.tile([128, 128], f32, tag="sig")
                nc.scalar.activation(sg[:], val[:, g * 128:(g + 1) * 128],
                                     mybir.ActivationFunctionType.Sigmoid)
                tp = pst.tile([128, 128], f32, tag="tp", space="PSUM")
                nc.tensor.transpose(tp[:], in_=sg[:], identity=ident[:])
                so = sm.tile([128, 128], f32, tag="sigT")
                nc.vector.tensor_copy(so[:], tp[:])
                nc.sync.dma_start(out_val[g * 128:(g + 1) * 128, :], so[:])

    nc.compile()
    return nc


_CACHE = {}


def kernel(z, edge_index, W1, b1, W2, b2):
    z = np.asarray(z, np.float32)
    edge_index = np.asarray(edge_index)
    W1 = np.asarray(W1, np.float32)
    W2 = np.asarray(W2, np.float32)
    b1 = np.asarray(b1, np.float32)
    b2 = np.asarray(b2, np.float32)
    if np.any(b1 != 0) or np.any(b2 != 0):
        return _host_reference(z, edge_index, W1, b1, W2, b2)

    from concourse import bass_utils

    plan = _plan(z, edge_index)
    key = (z.shape, edge_index.shape, plan['tot_agg'], plan['tot_sc'],
           tuple(plan['K'].tolist()), tuple(plan['SEG'].tolist()))
    if key not in _CACHE:
        _CACHE.clear()
        _CACHE[key] = _build(plan, W1, W2)
    nc = _CACHE[key]

    w1t = np.ascontiguousarray(W1.T)
    iota_row = np.tile(np.arange(128, dtype=np.float32), (128, 1)).astype(bf16)
    KMAX = plan['KMAX']
    iota_rep = np.tile(np.repeat(np.arange(128, dtype=np.float32), KMAX), (128, 1)).astype(bf16)
    in_maps = []
    for c in range(NC):
        in_maps.append({
            "z_cols_full": plan['z_cols_full'],
            "dinv_cols_full": plan['dinv_cols_full'],
            "z_cols": plan['z_cols'][c],
            "dinv_cols": plan['dinv_cols'][c],
            "dinv2_cols": plan['dinv2_cols'][c],
            "w1t": w1t, "w2": W2,
            "agg_idx": plan['agg_idx'][c],
            "agg_dstloc": plan['agg_dstloc'][c],
            "sc_v": plan['sc_v'][c],
            "sc_u": plan['sc_u'][c],
            "iota_row": iota_row,
            "iota_rep": iota_rep,
        })
    res = bass_utils.run_bass_kernel_spmd(nc, in_maps, core_ids=list(range(NC)))
    kernel._last = (nc, in_maps, plan)

    E = plan['E']
    flat = np.stack([res.results[c]["out_val"].reshape(-1) for c in range(NC)])
    out = flat[plan['sc_core'], plan['sc_slot']].astype(np.float32)
    return out
